# revision 11
# baseline (speedup 1.0000x reference)
"""Trainium2 Bass kernel for nn_MultiHeadAttention_738734375358.

Reference computation (per batch b):
    q = (x @ Wq + bq).reshape(n, H, HD)
    k, v = split((y @ Wkv + bkv).reshape(m, 2, H, HD))
    attn = softmax(q.k^T * HD^-0.5, axis=m)        -> output [b, n, m, h]
    out  = (attn @ v).reshape(n, C) @ Wp + bp      -> output [b, n, c]

Sharding: 8 cores = 2 batches x 4 head-groups (4 heads each).
Each core computes, for its (b, head-group):
  - qT [256, n], kT [256, m] (transposed projections, head dim on partitions)
  - v   [m, 4, 65]  (natural layout + ones column for softmax denominators)
  - per (n-block, head): scores^T tiles [m-tile, n-block] on PE, exp on ACT,
    attnv^T + denominators via ones-augmented matmul on PE,
    normalized attention written to DRAM as per-head [m, n] slabs,
  - partial out-projection [n, C] (summed over head-groups on host).

Matmuls run as float32r (fp32 storage, fp22 multiply) at full PE speed.
Host assembles: out = sum of partials + bp; attn = permuted slabs.
"""

import sys

if "/opt/trn_rl_repo" not in sys.path:
    sys.path.insert(0, "/opt/trn_rl_repo")

import numpy as np

# ---- problem constants (hardcoded per contest contract) ----
B, N, M, C, H, HD = 2, 2048, 2048, 1024, 16, 64
SCALE = HD ** (-0.5)  # 0.125
NCORES = 8
HPC = 4  # heads per core
DH = HPC * HD  # 256: projected cols per core
P = 128
CT = C // P  # 8 c-tiles
NT = N // P  # 16
MT = M // P  # 16

_CACHE = {}


def _build_program():
    import concourse.bass as bass
    import concourse.tile as tile
    from concourse import mybir
    from concourse.masks import make_identity

    f32 = mybir.dt.float32
    f32r = mybir.dt.float32r
    AF = mybir.ActivationFunctionType

    nc = bass.Bass()
    x = nc.declare_dram_parameter("x", [N, C], f32, isOutput=False)
    y = nc.declare_dram_parameter("y", [M, C], f32, isOutput=False)
    wq = nc.declare_dram_parameter("wq", [C, DH], f32, isOutput=False)
    wk = nc.declare_dram_parameter("wk", [C, DH], f32, isOutput=False)
    wv = nc.declare_dram_parameter("wv", [C, DH], f32, isOutput=False)
    wp = nc.declare_dram_parameter("wp", [DH, C], f32, isOutput=False)
    bq = nc.declare_dram_parameter("bq", [DH], f32, isOutput=False)
    bk = nc.declare_dram_parameter("bk", [DH], f32, isOutput=False)
    bv = nc.declare_dram_parameter("bv", [DH], f32, isOutput=False)
    attn_t = nc.declare_dram_parameter("attn_t", [HPC, M, N], f32, isOutput=True)
    outp = nc.declare_dram_parameter("outp", [N, C], f32, isOutput=True)

    with tile.TileContext(nc) as tc:
        with (
            tc.tile_pool(name="const", bufs=1) as const,
            tc.tile_pool(name="wpool", bufs=2) as wpool,
            tc.tile_pool(name="actT", bufs=1) as actTp,
            tc.tile_pool(name="natp", bufs=3) as natp,
            tc.tile_pool(name="qTp", bufs=1) as qTp,
            tc.tile_pool(name="kTp", bufs=1) as kTp,
            tc.tile_pool(name="vstp", bufs=1) as vstp,
            tc.tile_pool(name="work", bufs=28) as work,
            tc.tile_pool(name="psA", bufs=3, space="PSUM") as psA,
            tc.tile_pool(name="psB", bufs=2, space="PSUM") as psB,
            tc.tile_pool(name="psC", bufs=1, space="PSUM") as psC,
            tc.tile_pool(name="psT", bufs=2, space="PSUM") as psT,
        ):
            ident = const.tile([P, P], f32)
            make_identity(nc, ident)

            ones_row = const.tile([1, P], f32)
            nc.vector.memset(ones_row, 1.0)

            # bias columns [128, 2]: chunk ch has dims ch*128..ch*128+127
            bq_c = const.tile([P, 2], f32)
            nc.sync.dma_start(out=bq_c, in_=bq[:].rearrange("(c p) -> p c", p=P))
            bk_c = const.tile([P, 2], f32)
            nc.sync.dma_start(out=bk_c, in_=bk[:].rearrange("(c p) -> p c", p=P))
            bv_c = const.tile([P, 2], f32)
            nc.sync.dma_start(out=bv_c, in_=bv[:].rearrange("(c p) -> p c", p=P))

            # Wp rows per head: wp_sb[p, h, c] = wp[h*64 + p, c]
            wp_sb = const.tile([64, HPC, C], f32r)
            nc.sync.dma_start(
                out=wp_sb, in_=wp[:].rearrange("(h p) c -> p h c", p=64).bitcast(f32r)
            )

            # persistent projected tensors
            qT = qTp.tile([P, 2, N], f32r)  # [dq-in-chunk, chunk, n]
            kT = kTp.tile([P, 2, M], f32r)
            # v natural + ones col: v_store[p, mt, h, 0:64] = v[mt*128+p, h*64:...]
            v_store = vstp.tile([P, MT, HPC, 65], f32r)
            nc.vector.memset(v_store[:, :, :, 64:65].bitcast(f32), 1.0)

            def transpose_half(src_dram, half, dst_actT):
                """Transpose rows [half*1024, half*1024+1024) of src [2048, 1024]
                into dst_actT [128, 8(ct), 1024] (f32r)."""
                for lt in range(8):  # 128-row tiles within the half
                    rt = half * 8 + lt
                    nat = natp.tile([P, C], f32, tag="nat")
                    nc.sync.dma_start(
                        out=nat, in_=src_dram[rt * P : (rt + 1) * P, :]
                    )
                    for ct in range(CT):
                        pst = psT.tile([P, P], f32, tag="t")
                        nc.tensor.transpose(
                            pst, nat[:, ct * P : (ct + 1) * P], ident
                        )
                        nc.scalar.copy(
                            out=dst_actT[:, ct, lt * P : (lt + 1) * P], in_=pst
                        )

            def proj_half(w_sb, bias_c, half, dst, dst_is_vtmp):
                """Accumulate dst[:, ch, half*1024 + lb*512 ...] = w_sb[ct,ch].T @ actT
                over ct, add bias; dst [128, 2, 2048] (qT/kT) or via v-tmp path."""
                for ch in range(2):
                    for lb in range(2):  # local 512-blocks within the half
                        ps = psA.tile([P, 512], f32, tag="s")
                        for ct in range(CT):
                            nc.tensor.matmul(
                                ps,
                                lhsT=w_sb[:, ct, ch * P : (ch + 1) * P],
                                rhs=actT[:, ct, lb * 512 : (lb + 1) * 512],
                                start=(ct == 0),
                                stop=(ct == CT - 1),
                            )
                        if not dst_is_vtmp:
                            off = half * 1024 + lb * 512
                            nc.scalar.activation(
                                out=dst[:, ch, off : off + 512],
                                in_=ps,
                                func=AF.Identity,
                                bias=bias_c[:, ch : ch + 1],
                                scale=1.0,
                            )
                        else:
                            # v path: evac chunk [128(dv), 512(m)] then PE-transpose
                            # each head-64 x m-128 block into v_store
                            vtmp = work.tile([P, 512], f32r, tag="wk")
                            nc.scalar.activation(
                                out=vtmp,
                                in_=ps,
                                func=AF.Identity,
                                bias=bias_c[:, ch : ch + 1],
                                scale=1.0,
                            )
                            for hh in range(2):
                                h = 2 * ch + hh
                                hsl = slice(hh * 64, hh * 64 + 64)
                                for lm in range(4):  # m-tiles inside 512
                                    mt = half * 8 + lb * 4 + lm
                                    pst = psT.tile([P, 64], f32, tag="t")
                                    nc.tensor.transpose(
                                        pst,
                                        vtmp[hsl, lm * P : (lm + 1) * P].bitcast(f32),
                                        ident[hsl, hsl],
                                    )
                                    nc.scalar.copy(
                                        out=v_store[:, mt, h, 0:64], in_=pst
                                    )

            # ---- phase 1: x -> xT -> qT ----
            wq_sb = wpool.tile([P, CT, DH], f32r, tag="w")
            nc.sync.dma_start(
                out=wq_sb, in_=wq[:].rearrange("(t p) m -> p t m", p=P).bitcast(f32r)
            )
            for half in range(2):
                actT = actTp.tile([P, CT, 1024], f32r, tag="actT")
                transpose_half(x, half, actT)
                proj_half(wq_sb, bq_c, half, qT, False)

            # ---- phase 2: y -> yT -> kT and v ----
            wk_sb = wpool.tile([P, CT, DH], f32r, tag="w")
            nc.sync.dma_start(
                out=wk_sb, in_=wk[:].rearrange("(t p) m -> p t m", p=P).bitcast(f32r)
            )
            wv_sb = wpool.tile([P, CT, DH], f32r, tag="w")
            nc.sync.dma_start(
                out=wv_sb, in_=wv[:].rearrange("(t p) m -> p t m", p=P).bitcast(f32r)
            )
            for half in range(2):
                actT = actTp.tile([P, CT, 1024], f32r, tag="actT")
                transpose_half(y, half, actT)
                proj_half(wk_sb, bk_c, half, kT, False)
                proj_half(wv_sb, bv_c, half, None, True)

            # ---- phase 3: attention per (n-block, head) ----
            import os

            _phases = os.environ.get("KERNEL_PHASES", "all")
            _nb_range = {"all": 4, "attn1": 1, "proj": 0}.get(_phases, 4)
            for nb in range(_nb_range):
                nsl = slice(nb * 512, (nb + 1) * 512)
                av_tiles = []
                for h in range(HPC):
                    ch = h // 2
                    ksl = slice((h % 2) * 64, (h % 2) * 64 + 64)
                    ps_av = psB.tile([P, 512], f32, tag="av")
                    pts = []
                    for mt in range(MT):
                        ps_s = psA.tile([P, 512], f32, tag="s")
                        nc.tensor.matmul(
                            ps_s,
                            lhsT=kT[ksl, ch, mt * P : (mt + 1) * P],
                            rhs=qT[ksl, ch, nsl],
                            start=True,
                            stop=True,
                        )
                        pt = work.tile([P, 512], f32r, tag="wk")
                        nc.scalar.activation(
                            out=pt, in_=ps_s, func=AF.Exp, scale=SCALE
                        )
                        pts.append(pt)
                        nc.tensor.matmul(
                            ps_av[0:65, :],
                            lhsT=v_store[:, mt, h, :],
                            rhs=pt,
                            start=(mt == 0),
                            stop=(mt == MT - 1),
                        )
                    # denominators -> reciprocal -> broadcast across partitions
                    # (K=1 ones-matmul: rdb_ps[p, i] = rd[0, i] for all p)
                    rd = work.tile([1, 512], f32, tag="wk")
                    nc.vector.reciprocal(out=rd, in_=ps_av[64:65, :])
                    rdb_ps = psC.tile([P, 512], f32, tag="o")
                    nc.tensor.matmul(
                        rdb_ps, lhsT=ones_row, rhs=rd, start=True, stop=True
                    )
                    rdb = work.tile([P, 512], f32, tag="wk")
                    nc.scalar.copy(out=rdb, in_=rdb_ps)
                    # normalized attnv^T for this head
                    av = work.tile([64, 512], f32r, tag="wk")
                    nc.vector.tensor_mul(av, ps_av[0:64, :], rdb[0:64, :])
                    av_tiles.append(av)
                    # normalized attention rows -> DRAM (per-head [m, n] slab)
                    for mt in range(MT):
                        aw = work.tile([P, 512], f32, tag="wk")
                        nc.vector.tensor_mul(aw, pts[mt].bitcast(f32), rdb)
                        nc.sync.dma_start(
                            out=attn_t[h, mt * P : (mt + 1) * P, nsl], in_=aw
                        )
                # out-projection for this n-block (accumulate heads)
                for nt in range(4 if _phases != "noout" else 0):
                    for cb in range(2):
                        ps_o = psC.tile([P, 512], f32, tag="o")
                        for h in range(HPC):
                            nc.tensor.matmul(
                                ps_o,
                                lhsT=av_tiles[h][:, nt * P : (nt + 1) * P],
                                rhs=wp_sb[:, h, cb * 512 : (cb + 1) * 512],
                                start=(h == 0),
                                stop=(h == HPC - 1),
                            )
                        o_sb = work.tile([P, 512], f32, tag="wk")
                        nc.scalar.copy(out=o_sb, in_=ps_o)
                        nc.sync.dma_start(
                            out=outp[
                                nb * 512 + nt * P : nb * 512 + (nt + 1) * P,
                                cb * 512 : (cb + 1) * 512,
                            ],
                            in_=o_sb,
                        )

    import bass_rust

    bass_rust.generate_event_semaphores(nc)
    nc.finalize()
    return nc


def _get_program():
    if "nc" not in _CACHE:
        _CACHE["nc"] = _build_program()
    return _CACHE["nc"]


def _shard_inputs(x, y, Wq, bq, Wkv, bkv, Wp, bp):
    x = np.ascontiguousarray(x, dtype=np.float32)
    y = np.ascontiguousarray(y, dtype=np.float32)
    Wq = np.asarray(Wq, dtype=np.float32)
    Wkv = np.asarray(Wkv, dtype=np.float32)
    Wp = np.asarray(Wp, dtype=np.float32)
    bq = np.asarray(bq, dtype=np.float32)
    bkv = np.asarray(bkv, dtype=np.float32)
    in_maps = []
    for d in range(NCORES):
        b, g = d // HPC, d % HPC
        S = slice(DH * g, DH * (g + 1))
        in_maps.append(
            {
                "x": x[b],
                "y": y[b],
                "wq": np.ascontiguousarray(Wq[:, S]),
                "wk": np.ascontiguousarray(Wkv[:, S]),
                "wv": np.ascontiguousarray(Wkv[:, C:][:, S]),
                "wp": np.ascontiguousarray(Wp[S, :]),
                "bq": np.ascontiguousarray(bq[S]),
                "bk": np.ascontiguousarray(bkv[S]),
                "bv": np.ascontiguousarray(bkv[C + DH * g : C + DH * (g + 1)]),
            }
        )
    return in_maps


def _assemble(results, bp):
    out = np.empty((B, N, C), np.float32)
    bp = np.asarray(bp, dtype=np.float32)
    A = np.empty((B, H, M, N), np.float32)
    for b in range(B):
        acc = None
        for g in range(HPC):
            o = results[b * HPC + g]["outp"]
            acc = o.copy() if acc is None else acc + o
            A[b, HPC * g : HPC * (g + 1)] = results[b * HPC + g]["attn_t"]
        out[b] = acc + bp[None, :]
    # [b, h, m, n] -> [b, n, m, h] as a strided view (no 512MB permute copy)
    attn = A.transpose(0, 3, 2, 1)
    return out, attn


def kernel(x, y, Wq, bq, Wkv, bkv, Wp, bp):
    from concourse.bass_utils import run_bass_kernel_spmd

    nc = _get_program()
    in_maps = _shard_inputs(x, y, Wq, bq, Wkv, bkv, Wp, bp)
    res = run_bass_kernel_spmd(nc, in_maps, list(range(NCORES)))
    return _assemble(res.results, bp)
